# revision 32
# baseline (speedup 1.0000x reference)
"""GCN encoder (3x GCNConv) Trainium2 Bass kernel, 8-core SPMD.

Strategy (dst-sharded message passing):
- Nodes dst-sharded across 8 cores (12544-row padded shards). Each core owns
  all edges (incl. self-loops) whose dst lands in its shard.
- Activations are kept as T' = dis * (H @ W) in fp16, replicated in DRAM via
  AllGather after each layer's transform.
- Propagate per core: for each 128-dst block, gather T'[src] rows via
  gpsimd.dma_gather (int16 indices => T_full split into 4 row-chunks).
  Gather calls round-robin all 4 SWDGE queues with deep msg pools so
  descriptor generation overlaps across queues (~2-4ns/desc vs 8.4 serial).
- Slot regions use RAW per-(chunk,block) capacities (max over cores, no
  128-rounding); only gather-call boundaries are padded (16 for the idx AP,
  call starts 128-aligned in the msg tile). Segment matmuls run on
  partition-offset pieces, splitting at 128-partition wraps.
- Routing tiles oh[e,d] = (dstl[e]==d) are pure 0/1, built 8 tiles per DVE
  op via broadcast APs (iota [128,1,128] is_equal dstl [128,8,1]).
- All layers accumulate psum[d,f] = oh.T @ msg (+ identity @ ownblock for
  the self-loops, + sqrtdeg x b outer product for the bias). dis[src] rides
  in T' rows; dis[dst] is the per-partition fp32 activation scale at evac:
  relu(dis*(raw + sqrtdeg*b)) == relu(dis*raw + b) since dis*sqrtdeg == 1.
- Layers 1-2 then TensorE-transpose h to hT = lhsT of the transform GEMM
  T' = dis*(h @ W); layer 3 evacuates fp32 node-major output directly.
- AllGathers are split into half-shard collectives on separate DRAM
  tensors: half A is emitted mid-propagate (after block 48), half B between
  layers, hidden behind the next layer's chunk-0/1 gathers which only need
  half A and are emitted ahead of any chunk-2/3 call.
"""

import sys
import numpy as np

for _p in ("/opt/trn_rl_repo", "/root/.axon_site/_ro/trn_rl_repo"):
    if _p not in sys.path:
        sys.path.append(_p)

N_NODES = 100000
N_FEAT = 4
D = 128
NC = 8
NCHUNK = 4
GBLK = 3  # blocks per gather group
OHB = 8  # one-hot tiles built per DVE op
MAXIDX = 1024  # max indices per dma_gather call (SWDGE desc ring capacity)

f16 = np.float16


# ---------------------------------------------------------------- host side


def _cfg(n_nodes):
    nshard = (n_nodes + NC - 1) // NC
    shpad = ((nshard + 127) // 128) * 128
    nblk = shpad // 128
    nfull = NC * shpad
    assert nfull % NCHUNK == 0
    chunk = nfull // NCHUNK
    assert chunk <= 32767 + 1  # int16 index reach (idx < chunk <= 32768)
    return dict(n=n_nodes, nshard=nshard, shpad=shpad, nblk=nblk,
                nfull=nfull, chunk=chunk)


def _groups(nblk):
    return [(g, min(g + GBLK, nblk)) for g in range(0, nblk, GBLK)]


def _build_schedule(cfg, edge_index):
    """Integer/index preprocessing.

    Slot geometry: logical slots (descriptor/idx space, call-contiguous,
    16-aligned call sizes) vs device slots (msg-tile/one-hot column space,
    128-aligned call starts; gather-call tails are never written and get
    dstl=-1 so their one-hot rows are zero).
    """
    n, nshard, shpad, nblk, chunk = (cfg[k] for k in
                                     ("n", "nshard", "shpad", "nblk", "chunk"))
    # deg/dis include the added self-loops, but the self-loop edges
    # themselves are handled on-device by a diagonal matmul against the
    # core's own T' block (affine DMA, no gather descriptors).
    deg = np.bincount(np.concatenate([edge_index[1], np.arange(n)]),
                      minlength=n).astype(np.int64)
    dis = np.where(deg > 0, 1.0 / np.sqrt(deg.astype(np.float64)), 0.0)
    src = edge_index[0].astype(np.int64)
    dst = edge_index[1].astype(np.int64)

    # row in the split T_full layout: top half-shards of all cores first
    # (tfullA = rows [0, NC*shpad/2)), then bottom half-shards (tfullB).
    half = shpad // 2
    core_of = src // nshard
    local = src % nshard
    rows = np.where(local < half,
                    core_of * half + local,
                    NC * half + core_of * half + (local - half))
    echunk = rows // chunk
    ecore = dst // nshard
    eblk = (dst % nshard) // 128
    edstl = (dst % nshard) % 128

    counts = np.zeros((NC, NCHUNK, nblk), dtype=np.int64)
    np.add.at(counts, (ecore, echunk, eblk), 1)
    # 32-aligned so every region starts on a PE quadrant boundary
    cap = np.maximum(((counts.max(axis=0) + 31) // 32) * 32, 32)

    # slot layout: group -> chunk -> packed whole regions per call.
    rlog = np.zeros((NCHUNK, nblk), dtype=np.int64)
    rdev = np.zeros((NCHUNK, nblk), dtype=np.int64)
    calls = []  # (chunk, group_index, log_off, dev_off, nslots)
    off_log = 0
    off_dev = 0

    def close_call(c, gi, log0, dev0):
        nonlocal off_log, off_dev
        pad16 = (-(off_log - log0)) % 16
        off_log += pad16
        off_dev += pad16
        nslots = off_log - log0
        if nslots:
            calls.append((c, gi, log0, dev0, nslots))
        off_dev = dev0 + ((off_dev - dev0 + 127) // 128) * 128

    for gi, (blo, bhi) in enumerate(_groups(nblk)):
        for c in range(NCHUNK):
            log0, dev0 = off_log, off_dev
            for b in range(blo, bhi):
                if off_log + int(cap[c, b]) - log0 > MAXIDX:
                    close_call(c, gi, log0, dev0)
                    log0, dev0 = off_log, off_dev
                rlog[c, b] = off_log
                rdev[c, b] = off_dev
                off_log += int(cap[c, b])
                off_dev += int(cap[c, b])
            close_call(c, gi, log0, dev0)
    log_total = off_log
    dev_total = off_dev
    assert log_total % 16 == 0 and dev_total % 128 == 0

    cores = []
    for ci in range(NC):
        m = ecore == ci
        r, ec, eb, dl, dd = rows[m], echunk[m], eblk[m], edstl[m], dst[m]
        order = np.lexsort((r, eb, ec))
        r, ec, eb, dl, dd = (a[order] for a in (r, ec, eb, dl, dd))
        key = ec * nblk + eb
        starts = np.searchsorted(key, np.arange(NCHUNK * nblk))
        ends = np.searchsorted(key, np.arange(NCHUNK * nblk), side="right")

        idx = np.zeros(log_total, np.int64)
        dstl = np.full(dev_total, -1.0, np.float64)
        disdst = np.ones(dev_total, np.float64)
        for c in range(NCHUNK):
            for b in range(nblk):
                s, e = starts[c * nblk + b], ends[c * nblk + b]
                nn = e - s
                ol, od = rlog[c, b], rdev[c, b]
                assert nn <= cap[c, b]
                idx[ol:ol + nn] = r[s:e] % chunk
                idx[ol + nn:ol + cap[c, b]] = r[e - 1] % chunk if nn else 0
                dstl[od:od + nn] = dl[s:e]
                disdst[od:od + nn] = dis[dd[s:e]]
        # call pad16 tails keep idx=0 (a valid row; their device slots have
        # dstl=-1 so the one-hot zeroes the contribution)
        cores.append(dict(idx=idx.astype(np.int16),
                          dstl=dstl.astype(f16),
                          disdst=disdst.astype(f16)))

    return dis, cap, rlog, rdev, calls, log_total, dev_total, cores


# --------------------------------------------------------------- bass build


def _build_program(cfg, cap, rlog, rdev, calls, log_total, dev_total):
    import concourse.bacc as bacc
    import concourse.tile as tile
    from concourse import mybir

    nblk, shpad, nfull, chunk = (cfg[k] for k in
                                 ("nblk", "shpad", "nfull", "chunk"))
    dt = mybir.dt
    AF = mybir.ActivationFunctionType
    OP = mybir.AluOpType
    S_dev = dev_total // 128
    idxcols = log_total // 16
    groups = _groups(nblk)

    nc = bacc.Bacc("TRN2", target_bir_lowering=False, debug=False,
                   num_devices=NC, num_swdge_queues=4)

    # --- I/O
    xT_d = nc.dram_tensor("xT", [N_FEAT, shpad], dt.float16, kind="ExternalInput")
    W1_d = nc.dram_tensor("W1", [N_FEAT, D], dt.float16, kind="ExternalInput")
    W2_d = nc.dram_tensor("W2", [D, D], dt.float16, kind="ExternalInput")
    W3_d = nc.dram_tensor("W3", [D, D], dt.float16, kind="ExternalInput")
    b1r_d = nc.dram_tensor("b1r", [1, D], dt.float16, kind="ExternalInput")
    b2r_d = nc.dram_tensor("b2r", [1, D], dt.float16, kind="ExternalInput")
    b3r_d = nc.dram_tensor("b3r", [1, D], dt.float16, kind="ExternalInput")
    disc_d = nc.dram_tensor("disc", [128, nblk], dt.float32, kind="ExternalInput")
    sqd_d = nc.dram_tensor("sqd", [1, shpad], dt.float16, kind="ExternalInput")
    dstl_d = nc.dram_tensor("dstl", [128, S_dev], dt.float16, kind="ExternalInput")
    idx_d = nc.dram_tensor("idx16", [128, idxcols], dt.int16, kind="ExternalInput")
    iota_d = nc.dram_tensor("iota", [128, D], dt.float16, kind="ExternalInput")
    ident_d = nc.dram_tensor("ident", [128, D], dt.float16, kind="ExternalInput")
    ones_d = nc.dram_tensor("ones1", [1, D], dt.float16, kind="ExternalInput")
    out_d = nc.dram_tensor("out", [shpad, D], dt.float32, kind="ExternalOutput")

    # internal DRAM: allgather bounce + double-buffered replicated T', split
    # into top/bottom half-shard tensors so each half's AllGather can be
    # emitted as soon as its source blocks are evacuated and the next
    # layer's gathers can start on the half that's ready.
    half = shpad // 2
    hblk = half // 128
    tlocA = nc.dram_tensor("t_locA", [half, D], dt.float16)
    tlocB = nc.dram_tensor("t_locB", [half, D], dt.float16)
    tfA = [nc.dram_tensor(f"t_fullA{i}", [NC * half, D], dt.float16)
           for i in range(2)]
    tfB = [nc.dram_tensor(f"t_fullB{i}", [NC * half, D], dt.float16)
           for i in range(2)]

    def tloc_slice(b):
        if b < hblk:
            return tlocA[b * 128:(b + 1) * 128, :]
        return tlocB[(b - hblk) * 128:(b - hblk + 1) * 128, :]

    def emit_ag(nc_, mybir_, parity, which):
        src = tlocA if which == 0 else tlocB
        dst = (tfA if which == 0 else tfB)[parity]
        nc_.gpsimd.collective_compute(
            "AllGather", mybir_.AluOpType.bypass,
            replica_groups=[list(range(NC))],
            ins=[src[:, :].opt()], outs=[dst[:, :].opt()])

    # per-(group, chunk) device-column extents for msg tiles
    gdev0 = {}
    gdevcols = {}
    for (c, gi, log0, dev0, nslots) in calls:
        k = (gi, c)
        if k not in gdev0:
            gdev0[k] = dev0
        gdevcols[k] = (dev0 - gdev0[k]) // 128 + (nslots + 127) // 128
    maxsub = {c: max(v for (gi, cc), v in gdevcols.items() if cc == c)
              for c in range(NCHUNK)}

    from contextlib import ExitStack
    with tile.TileContext(nc) as tc, ExitStack() as stack:
        # ---- resident tiles (pool stays open for the whole program)
        res = stack.enter_context(tc.tile_pool(name="res", bufs=1))
        idx_sb = res.tile([128, idxcols], dt.int16, tag="idx")
        dstl_sb = res.tile([128, S_dev], dt.float16, tag="dstl")
        sqd_sb = res.tile([1, shpad], dt.float16, tag="sqd")
        disc_sb = res.tile([128, nblk], dt.float32, tag="disc")
        iota_sb = res.tile([128, D], dt.float16, tag="iota")
        ident_sb = res.tile([128, D], dt.float16, tag="ident")
        ones_sb = res.tile([1, D], dt.float16, tag="ones")
        xT_sb = res.tile([N_FEAT, shpad], dt.float16, tag="xT")
        W1_sb = res.tile([N_FEAT, D], dt.float16, tag="W1")
        W2_sb = res.tile([D, D], dt.float16, tag="W2")
        W3_sb = res.tile([D, D], dt.float16, tag="W3")
        b1r_sb = res.tile([1, D], dt.float16, tag="b1r")
        b2r_sb = res.tile([1, D], dt.float16, tag="b2r")
        b3r_sb = res.tile([1, D], dt.float16, tag="b3r")

        for sb, d in ((idx_sb, idx_d), (dstl_sb, dstl_d), (sqd_sb, sqd_d),
                      (disc_sb, disc_d), (iota_sb, iota_d), (ident_sb, ident_d),
                      (ones_sb, ones_d), (xT_sb, xT_d), (W1_sb, W1_d),
                      (W2_sb, W2_d), (W3_sb, W3_d), (b1r_sb, b1r_d),
                      (b2r_sb, b2r_d), (b3r_sb, b3r_d)):
            nc.sync.dma_start(out=sb[:], in_=d[:, :])

        # ---- layer 1 transform: T1' = dis * (x @ W1) -> tloc, allgather
        with (
            tc.tile_pool(name="p1ps", bufs=4, space="PSUM") as p1ps,
            tc.tile_pool(name="p1sb", bufs=4) as p1sb,
        ):
            for b in range(nblk):
                ps = p1ps.tile([128, D], dt.float32, tag="t1ps")
                nc.tensor.matmul(ps[:], xT_sb[:, b * 128:(b + 1) * 128],
                                 W1_sb[:], start=True, stop=True)
                t1 = p1sb.tile([128, D], dt.float16, tag="t1sb")
                nc.scalar.activation(t1[:], ps[:], AF.Copy,
                                     scale=disc_sb[:, b:b + 1])
                nc.sync.dma_start(out=tloc_slice(b), in_=t1[:])
                if b == hblk - 1:
                    emit_ag(nc, mybir, 0, 0)
        emit_ag(nc, mybir, 0, 1)

        # ---- layers
        qctr = 0
        with (
            tc.tile_pool(name="msgp", bufs=7) as msgp,
            tc.tile_pool(name="ohp", bufs=6) as ohp,

            tc.tile_pool(name="evp", bufs=4) as evp,
            tc.tile_pool(name="slp", bufs=4) as slp,
            tc.tile_pool(name="psp", bufs=4, space="PSUM") as psp,
            tc.tile_pool(name="ps2p", bufs=2, space="PSUM") as ps2p,
            tc.tile_pool(name="ps3p", bufs=2, space="PSUM") as ps3p,
        ):
          for layer in range(3):
            last = layer == 2

            def tsrc_view(c, _p=layer % 2):
                if c < 2:
                    return tfA[_p][c * chunk:(c + 1) * chunk, :]
                return tfB[_p][(c - 2) * chunk:(c - 1) * chunk, :]

            W_next = W2_sb if layer == 0 else W3_sb
            brow = (b1r_sb, b2r_sb, b3r_sb)[layer]
            if True:
                # gather emission leads consumption by EARLY groups; at
                # layer start the chunk-0/1 calls (fed by the already-done
                # AllGather half A) are emitted before any chunk-2/3 call so
                # the in-order GpSimd stream is not stalled by AG half B.
                EARLY = 5
                mtiles = {}
                lgroups = len(groups)

                def alloc_group(gi2):
                    for c in range(NCHUNK):
                        mt = msgp.tile([128, maxsub[c] * D], dt.float16,
                                       tag=f"msg{c}")
                        if layer == 0 and gi2 < 7:
                            # first rotation of each buffer: clear so call
                            # tails can never be NaN garbage in the matmuls
                            nc.vector.memset(mt[:], 0.0)
                        mtiles[(gi2, c)] = (mt, gdev0[(gi2, c)])

                def emit_calls(gi2, chunks):
                    nonlocal qctr
                    gcalls = {c: [cl for cl in calls
                                  if cl[0] == c and cl[1] == gi2]
                              for c in chunks}
                    mxcall = max(len(v) for v in gcalls.values())
                    for k in range(mxcall):
                        for c in chunks:
                            if k >= len(gcalls[c]):
                                continue
                            (_, _, log0, dev0, nslots) = gcalls[c][k]
                            mt, gbase = mtiles[(gi2, c)]
                            nsub = (nslots + 127) // 128
                            fo = (dev0 - gbase) // 128
                            nc.gpsimd.dma_gather(
                                mt[:, fo * D:(fo + nsub) * D]
                                .rearrange("p (s e) -> p s e", e=D),
                                tsrc_view(c),
                                idx_sb[:, log0 // 16:(log0 + nslots) // 16],
                                nslots, nslots, D, queue_num=qctr % 4)
                            qctr += 1

                for gi2 in range(min(EARLY, lgroups)):
                    alloc_group(gi2)
                for gi2 in range(min(EARLY, lgroups)):
                    emit_calls(gi2, (0, 1))
                for gi2 in range(min(EARLY, lgroups)):
                    emit_calls(gi2, (2, 3))

                for gi, (blo, bhi) in enumerate(groups):
                    if gi + EARLY < lgroups:
                        alloc_group(gi + EARLY)
                        emit_calls(gi + EARLY, (0, 1, 2, 3))

                    # --- one-hot tiles for the whole group, OHB per DVE op.
                    # Device tile order is (group, chunk, block, sub), so the
                    # group's tiles occupy contiguous dstl/disdst columns.
                    t0 = gdev0[(gi, 0)] // 128
                    t1 = (gdev0[(gi, NCHUNK - 1)] // 128
                          + gdevcols[(gi, NCHUNK - 1)])
                    ohtiles = {}
                    for tb in range(t0, t1, OHB):
                        nb = min(OHB, t1 - tb)
                        ohb = ohp.tile([128, nb, D], dt.float16, tag="ohb")
                        nc.vector.tensor_tensor(
                            ohb[:],
                            iota_sb[:].rearrange("p (s e) -> p s e", s=1)
                            .broadcast_to((128, nb, D)),
                            dstl_sb[:, tb:tb + nb]
                            .rearrange("p (s e) -> p s e", e=1)
                            .broadcast_to((128, nb, D)),
                            OP.is_equal)
                        for j in range(nb):
                            ohtiles[tb + j] = (ohb, j)

                    # --- segment-sum matmul pieces + evac per block
                    for b in range(blo, bhi):
                        st = slp.tile([128, D], dt.float16, tag="st")
                        nc.sync.dma_start(out=st[:], in_=tloc_slice(b))
                        # enumerate partition-aligned pieces over all chunks
                        pieces = []
                        for c in range(NCHUNK):
                            mt, gbase = mtiles[(gi, c)]
                            L = int(rdev[c, b]) - gbase
                            cnt = int(cap[c, b])
                            while cnt > 0:
                                p0 = L % 128
                                g = L // 128
                                # PE tile_position rule: start 0 -> up to
                                # 128 rows, start 64 -> 64, start 32/96 -> 32
                                K = min(128 if p0 == 0 else
                                        64 if p0 == 64 else 32, cnt)
                                pieces.append((mt, gbase, p0, g, K))
                                L += K
                                cnt -= K
                        ps = psp.tile([128, D], dt.float32, tag="ps")
                        for k, (mt, gbase, p0, g, K) in enumerate(pieces):
                            ohb, j = ohtiles[gbase // 128 + g]
                            oh = ohb[p0:p0 + K, j, :]
                            msl = mt[p0:p0 + K, g * D:(g + 1) * D]
                            nc.tensor.matmul(ps[:], oh, msl,
                                             start=(k == 0), stop=False)
                        nc.tensor.matmul(ps[:], ident_sb[:], st[:],
                                         start=False, stop=False)
                        # bias: ps += sqrtdeg[d] * b[f]; the dis scale at evac
                        # turns it into +b exactly (dis * sqrtdeg == 1)
                        nc.tensor.matmul(ps[:], sqd_sb[:, b * 128:(b + 1) * 128],
                                         brow[:], start=False, stop=True)
                        if last:
                            ot = evp.tile([128, D], dt.float32, tag="outsb")
                            nc.scalar.activation(ot[:], ps[:], AF.Copy,
                                                 scale=disc_sb[:, b:b + 1])
                            nc.sync.dma_start(
                                out=out_d[b * 128:(b + 1) * 128, :], in_=ot[:])
                        else:
                            h = evp.tile([128, D], dt.float16, tag="h")
                            nc.scalar.activation(h[:], ps[:], AF.Relu,
                                                 scale=disc_sb[:, b:b + 1])
                            ps3 = ps3p.tile([128, D], dt.float16, tag="ps3")
                            nc.tensor.transpose(ps3[:], h[:], ident_sb[:])
                            hT = evp.tile([128, D], dt.float16, tag="hT")
                            nc.scalar.activation(hT[:], ps3[:], AF.Copy)
                            ps2 = ps2p.tile([128, D], dt.float32, tag="ps2")
                            nc.tensor.matmul(ps2[:], hT[:], W_next[:],
                                             start=True, stop=True)
                            tn = evp.tile([128, D], dt.float16, tag="tn")
                            nc.scalar.activation(tn[:], ps2[:], AF.Copy,
                                                 scale=disc_sb[:, b:b + 1])
                            nc.sync.dma_start(out=tloc_slice(b), in_=tn[:])
                            if b == hblk - 1:
                                emit_ag(nc, mybir, (layer + 1) % 2, 0)
                if not last:
                    emit_ag(nc, mybir, (layer + 1) % 2, 1)

    nc.compile()
    return nc


# ------------------------------------------------------------------ driver


def _make_in_maps(cfg, dis, cores, inputs):
    n, nshard, shpad, nblk = (cfg[k] for k in ("n", "nshard", "shpad", "nblk"))
    x = np.asarray(inputs["x"], np.float32)
    W1 = np.asarray(inputs["W1"], f16)
    W2 = np.asarray(inputs["W2"], f16)
    W3 = np.asarray(inputs["W3"], f16)
    b1r = np.asarray(inputs["b1"], f16).reshape(1, D)
    b2r = np.asarray(inputs["b2"], f16).reshape(1, D)
    b3r = np.asarray(inputs["b3"], f16).reshape(1, D)
    iota = np.broadcast_to(np.arange(D, dtype=f16), (128, D)).copy()
    ident = np.eye(128, dtype=f16)
    ones1 = np.ones((1, D), f16)

    in_maps = []
    for ci in range(NC):
        xs = np.zeros((shpad, N_FEAT), np.float32)
        lo = min(ci * nshard, n)
        hi = min((ci + 1) * nshard, n)
        xs[:hi - lo] = x[lo:hi]
        diss = np.ones(shpad, np.float32)
        diss[:hi - lo] = dis[lo:hi]
        ca = cores[ci]
        log_total = len(ca["idx"])
        dev_total = len(ca["dstl"])
        in_maps.append({
            "xT": np.ascontiguousarray(xs.T.astype(f16)),
            "W1": W1, "W2": W2, "W3": W3,
            "b1r": b1r, "b2r": b2r, "b3r": b3r,
            "disc": np.ascontiguousarray(diss.reshape(nblk, 128).T),
            "sqd": np.ascontiguousarray((1.0 / diss).reshape(1, shpad)
                                        .astype(f16)),
            "dstl": np.ascontiguousarray(
                ca["dstl"].reshape(dev_total // 128, 128).T),
            "idx16": np.ascontiguousarray(
                np.tile(ca["idx"].reshape(log_total // 16, 16).T, (8, 1))),
            "iota": iota, "ident": ident, "ones1": ones1,
        })
    return in_maps


def run(inputs, n_nodes=N_NODES, trace=False):
    cfg = _cfg(n_nodes)
    edge_index = np.asarray(inputs["edge_index"]).astype(np.int64)
    dis, cap, rlog, rdev, calls, log_total, dev_total, cores = \
        _build_schedule(cfg, edge_index)
    nc = _build_program(cfg, cap, rlog, rdev, calls, log_total, dev_total)
    in_maps = _make_in_maps(cfg, dis, cores, inputs)

    from concourse.bass_utils import run_bass_kernel_spmd
    res = run_bass_kernel_spmd(nc, in_maps, core_ids=list(range(NC)),
                               trace=trace)
    n, nshard = cfg["n"], cfg["nshard"]
    out = np.concatenate(
        [res.results[ci]["out"][:min((ci + 1) * nshard, n) - ci * nshard]
         for ci in range(NC)], axis=0)
    return out.astype(np.float32), res


def kernel(**inputs) -> np.ndarray:
    out, _ = run(inputs)
    return out


# revision 33
# speedup vs baseline: 1.0687x; 1.0687x over previous
"""GCN encoder (3x GCNConv) Trainium2 Bass kernel, 8-core SPMD.

Strategy (dst-sharded message passing):
- Nodes dst-sharded across 8 cores (12544-row padded shards). Each core owns
  all edges (incl. self-loops) whose dst lands in its shard.
- Activations are kept as T' = dis * (H @ W) in fp16, replicated in DRAM via
  AllGather after each layer's transform.
- Propagate per core: for each 128-dst block, gather T'[src] rows via
  gpsimd.dma_gather (int16 indices => T_full split into 4 row-chunks).
  Gather calls round-robin all 4 SWDGE queues with deep msg pools so
  descriptor generation overlaps across queues (~2-4ns/desc vs 8.4 serial).
- Slot regions use RAW per-(chunk,block) capacities (max over cores, no
  128-rounding); only gather-call boundaries are padded (16 for the idx AP,
  call starts 128-aligned in the msg tile). Segment matmuls run on
  partition-offset pieces, splitting at 128-partition wraps.
- Routing tiles oh[e,d] = (dstl[e]==d) are pure 0/1, built 8 tiles per DVE
  op via broadcast APs (iota [128,1,128] is_equal dstl [128,8,1]).
- All layers accumulate psum[d,f] = oh.T @ msg (+ identity @ ownblock for
  the self-loops, + sqrtdeg x b outer product for the bias). dis[src] rides
  in T' rows; dis[dst] is the per-partition fp32 activation scale at evac:
  relu(dis*(raw + sqrtdeg*b)) == relu(dis*raw + b) since dis*sqrtdeg == 1.
- Layers 1-2 then TensorE-transpose h to hT = lhsT of the transform GEMM
  T' = dis*(h @ W); layer 3 evacuates fp32 node-major output directly.
- AllGathers are split into half-shard collectives on separate DRAM
  tensors: half A is emitted mid-propagate (after block 48), half B between
  layers, hidden behind the next layer's chunk-0/1 gathers which only need
  half A and are emitted ahead of any chunk-2/3 call.
"""

import sys
import numpy as np

for _p in ("/opt/trn_rl_repo", "/root/.axon_site/_ro/trn_rl_repo"):
    if _p not in sys.path:
        sys.path.append(_p)

N_NODES = 100000
N_FEAT = 4
D = 128
NC = 8
NCHUNK = 4
GBLK = 2  # blocks per gather group
OHB = 8  # one-hot tiles built per DVE op
MAXIDX = 1024  # max indices per dma_gather call (SWDGE desc ring capacity)

f16 = np.float16


# ---------------------------------------------------------------- host side


def _cfg(n_nodes):
    nshard = (n_nodes + NC - 1) // NC
    shpad = ((nshard + 127) // 128) * 128
    nblk = shpad // 128
    nfull = NC * shpad
    assert nfull % NCHUNK == 0
    chunk = nfull // NCHUNK
    assert chunk <= 32767 + 1  # int16 index reach (idx < chunk <= 32768)
    return dict(n=n_nodes, nshard=nshard, shpad=shpad, nblk=nblk,
                nfull=nfull, chunk=chunk)


def _groups(nblk):
    return [(g, min(g + GBLK, nblk)) for g in range(0, nblk, GBLK)]


def _build_schedule(cfg, edge_index):
    """Integer/index preprocessing.

    Slot geometry: logical slots (descriptor/idx space, call-contiguous,
    16-aligned call sizes) vs device slots (msg-tile/one-hot column space,
    128-aligned call starts; gather-call tails are never written and get
    dstl=-1 so their one-hot rows are zero).
    """
    n, nshard, shpad, nblk, chunk = (cfg[k] for k in
                                     ("n", "nshard", "shpad", "nblk", "chunk"))
    # deg/dis include the added self-loops, but the self-loop edges
    # themselves are handled on-device by a diagonal matmul against the
    # core's own T' block (affine DMA, no gather descriptors).
    deg = np.bincount(np.concatenate([edge_index[1], np.arange(n)]),
                      minlength=n).astype(np.int64)
    dis = np.where(deg > 0, 1.0 / np.sqrt(deg.astype(np.float64)), 0.0)
    src = edge_index[0].astype(np.int64)
    dst = edge_index[1].astype(np.int64)

    # row in the split T_full layout: top half-shards of all cores first
    # (tfullA = rows [0, NC*shpad/2)), then bottom half-shards (tfullB).
    half = shpad // 2
    core_of = src // nshard
    local = src % nshard
    rows = np.where(local < half,
                    core_of * half + local,
                    NC * half + core_of * half + (local - half))
    echunk = rows // chunk
    ecore = dst // nshard
    eblk = (dst % nshard) // 128
    edstl = (dst % nshard) % 128

    counts = np.zeros((NC, NCHUNK, nblk), dtype=np.int64)
    np.add.at(counts, (ecore, echunk, eblk), 1)
    # 32-aligned so every region starts on a PE quadrant boundary
    cap = np.maximum(((counts.max(axis=0) + 31) // 32) * 32, 32)

    # slot layout: group -> chunk -> packed whole regions per call.
    rlog = np.zeros((NCHUNK, nblk), dtype=np.int64)
    rdev = np.zeros((NCHUNK, nblk), dtype=np.int64)
    calls = []  # (chunk, group_index, log_off, dev_off, nslots)
    off_log = 0
    off_dev = 0

    def close_call(c, gi, log0, dev0):
        nonlocal off_log, off_dev
        pad16 = (-(off_log - log0)) % 16
        off_log += pad16
        off_dev += pad16
        nslots = off_log - log0
        if nslots:
            calls.append((c, gi, log0, dev0, nslots))
        off_dev = dev0 + ((off_dev - dev0 + 127) // 128) * 128

    for gi, (blo, bhi) in enumerate(_groups(nblk)):
        for c in range(NCHUNK):
            log0, dev0 = off_log, off_dev
            for b in range(blo, bhi):
                if off_log + int(cap[c, b]) - log0 > MAXIDX:
                    close_call(c, gi, log0, dev0)
                    log0, dev0 = off_log, off_dev
                rlog[c, b] = off_log
                rdev[c, b] = off_dev
                off_log += int(cap[c, b])
                off_dev += int(cap[c, b])
            close_call(c, gi, log0, dev0)
    log_total = off_log
    dev_total = off_dev
    assert log_total % 16 == 0 and dev_total % 128 == 0

    cores = []
    for ci in range(NC):
        m = ecore == ci
        r, ec, eb, dl, dd = rows[m], echunk[m], eblk[m], edstl[m], dst[m]
        order = np.lexsort((r, eb, ec))
        r, ec, eb, dl, dd = (a[order] for a in (r, ec, eb, dl, dd))
        key = ec * nblk + eb
        starts = np.searchsorted(key, np.arange(NCHUNK * nblk))
        ends = np.searchsorted(key, np.arange(NCHUNK * nblk), side="right")

        idx = np.zeros(log_total, np.int64)
        dstl = np.full(dev_total, -1.0, np.float64)
        disdst = np.ones(dev_total, np.float64)
        for c in range(NCHUNK):
            for b in range(nblk):
                s, e = starts[c * nblk + b], ends[c * nblk + b]
                nn = e - s
                ol, od = rlog[c, b], rdev[c, b]
                assert nn <= cap[c, b]
                idx[ol:ol + nn] = r[s:e] % chunk
                idx[ol + nn:ol + cap[c, b]] = r[e - 1] % chunk if nn else 0
                dstl[od:od + nn] = dl[s:e]
                disdst[od:od + nn] = dis[dd[s:e]]
        # call pad16 tails keep idx=0 (a valid row; their device slots have
        # dstl=-1 so the one-hot zeroes the contribution)
        cores.append(dict(idx=idx.astype(np.int16),
                          dstl=dstl.astype(f16),
                          disdst=disdst.astype(f16)))

    return dis, cap, rlog, rdev, calls, log_total, dev_total, cores


# --------------------------------------------------------------- bass build


def _build_program(cfg, cap, rlog, rdev, calls, log_total, dev_total):
    import concourse.bacc as bacc
    import concourse.tile as tile
    from concourse import mybir

    nblk, shpad, nfull, chunk = (cfg[k] for k in
                                 ("nblk", "shpad", "nfull", "chunk"))
    dt = mybir.dt
    AF = mybir.ActivationFunctionType
    OP = mybir.AluOpType
    S_dev = dev_total // 128
    idxcols = log_total // 16
    groups = _groups(nblk)

    nc = bacc.Bacc("TRN2", target_bir_lowering=False, debug=False,
                   num_devices=NC, num_swdge_queues=4)

    # --- I/O
    xT_d = nc.dram_tensor("xT", [N_FEAT, shpad], dt.float16, kind="ExternalInput")
    W1_d = nc.dram_tensor("W1", [N_FEAT, D], dt.float16, kind="ExternalInput")
    W2_d = nc.dram_tensor("W2", [D, D], dt.float16, kind="ExternalInput")
    W3_d = nc.dram_tensor("W3", [D, D], dt.float16, kind="ExternalInput")
    b1r_d = nc.dram_tensor("b1r", [1, D], dt.float16, kind="ExternalInput")
    b2r_d = nc.dram_tensor("b2r", [1, D], dt.float16, kind="ExternalInput")
    b3r_d = nc.dram_tensor("b3r", [1, D], dt.float16, kind="ExternalInput")
    disc_d = nc.dram_tensor("disc", [128, nblk], dt.float32, kind="ExternalInput")
    sqd_d = nc.dram_tensor("sqd", [1, shpad], dt.float16, kind="ExternalInput")
    dstl_d = nc.dram_tensor("dstl", [128, S_dev], dt.float16, kind="ExternalInput")
    idx_d = nc.dram_tensor("idx16", [128, idxcols], dt.int16, kind="ExternalInput")
    iota_d = nc.dram_tensor("iota", [128, D], dt.float16, kind="ExternalInput")
    ident_d = nc.dram_tensor("ident", [128, D], dt.float16, kind="ExternalInput")
    ones_d = nc.dram_tensor("ones1", [1, D], dt.float16, kind="ExternalInput")
    out_d = nc.dram_tensor("out", [shpad, D], dt.float32, kind="ExternalOutput")

    # internal DRAM: allgather bounce + double-buffered replicated T', split
    # into top/bottom half-shard tensors so each half's AllGather can be
    # emitted as soon as its source blocks are evacuated and the next
    # layer's gathers can start on the half that's ready.
    half = shpad // 2
    hblk = half // 128
    tlocA = nc.dram_tensor("t_locA", [half, D], dt.float16)
    tlocB = nc.dram_tensor("t_locB", [half, D], dt.float16)
    tfA = [nc.dram_tensor(f"t_fullA{i}", [NC * half, D], dt.float16)
           for i in range(2)]
    tfB = [nc.dram_tensor(f"t_fullB{i}", [NC * half, D], dt.float16)
           for i in range(2)]

    def tloc_slice(b):
        if b < hblk:
            return tlocA[b * 128:(b + 1) * 128, :]
        return tlocB[(b - hblk) * 128:(b - hblk + 1) * 128, :]

    def emit_ag(nc_, mybir_, parity, which):
        src = tlocA if which == 0 else tlocB
        dst = (tfA if which == 0 else tfB)[parity]
        nc_.gpsimd.collective_compute(
            "AllGather", mybir_.AluOpType.bypass,
            replica_groups=[list(range(NC))],
            ins=[src[:, :].opt()], outs=[dst[:, :].opt()])

    # per-(group, chunk) device-column extents for msg tiles
    gdev0 = {}
    gdevcols = {}
    for (c, gi, log0, dev0, nslots) in calls:
        k = (gi, c)
        if k not in gdev0:
            gdev0[k] = dev0
        gdevcols[k] = (dev0 - gdev0[k]) // 128 + (nslots + 127) // 128
    maxsub = {c: max(v for (gi, cc), v in gdevcols.items() if cc == c)
              for c in range(NCHUNK)}

    from contextlib import ExitStack
    with tile.TileContext(nc) as tc, ExitStack() as stack:
        # ---- resident tiles (pool stays open for the whole program)
        res = stack.enter_context(tc.tile_pool(name="res", bufs=1))
        idx_sb = res.tile([128, idxcols], dt.int16, tag="idx")
        dstl_sb = res.tile([128, S_dev], dt.float16, tag="dstl")
        sqd_sb = res.tile([1, shpad], dt.float16, tag="sqd")
        disc_sb = res.tile([128, nblk], dt.float32, tag="disc")
        iota_sb = res.tile([128, D], dt.float16, tag="iota")
        ident_sb = res.tile([128, D], dt.float16, tag="ident")
        ones_sb = res.tile([1, D], dt.float16, tag="ones")
        xT_sb = res.tile([N_FEAT, shpad], dt.float16, tag="xT")
        W1_sb = res.tile([N_FEAT, D], dt.float16, tag="W1")
        W2_sb = res.tile([D, D], dt.float16, tag="W2")
        W3_sb = res.tile([D, D], dt.float16, tag="W3")
        b1r_sb = res.tile([1, D], dt.float16, tag="b1r")
        b2r_sb = res.tile([1, D], dt.float16, tag="b2r")
        b3r_sb = res.tile([1, D], dt.float16, tag="b3r")

        for sb, d in ((idx_sb, idx_d), (dstl_sb, dstl_d), (sqd_sb, sqd_d),
                      (disc_sb, disc_d), (iota_sb, iota_d), (ident_sb, ident_d),
                      (ones_sb, ones_d), (xT_sb, xT_d), (W1_sb, W1_d),
                      (W2_sb, W2_d), (W3_sb, W3_d), (b1r_sb, b1r_d),
                      (b2r_sb, b2r_d), (b3r_sb, b3r_d)):
            nc.sync.dma_start(out=sb[:], in_=d[:, :])

        # ---- layer 1 transform: T1' = dis * (x @ W1) -> tloc, allgather
        with (
            tc.tile_pool(name="p1ps", bufs=4, space="PSUM") as p1ps,
            tc.tile_pool(name="p1sb", bufs=4) as p1sb,
        ):
            for b in range(nblk):
                ps = p1ps.tile([128, D], dt.float32, tag="t1ps")
                nc.tensor.matmul(ps[:], xT_sb[:, b * 128:(b + 1) * 128],
                                 W1_sb[:], start=True, stop=True)
                t1 = p1sb.tile([128, D], dt.float16, tag="t1sb")
                nc.scalar.activation(t1[:], ps[:], AF.Copy,
                                     scale=disc_sb[:, b:b + 1])
                nc.sync.dma_start(out=tloc_slice(b), in_=t1[:])
                if b == hblk - 1:
                    emit_ag(nc, mybir, 0, 0)
        emit_ag(nc, mybir, 0, 1)

        # ---- layers
        qctr = 0
        with (
            tc.tile_pool(name="msgp", bufs=10) as msgp,
            tc.tile_pool(name="ohp", bufs=6) as ohp,

            tc.tile_pool(name="evp", bufs=4) as evp,
            tc.tile_pool(name="slp", bufs=4) as slp,
            tc.tile_pool(name="psp", bufs=4, space="PSUM") as psp,
            tc.tile_pool(name="ps2p", bufs=2, space="PSUM") as ps2p,
            tc.tile_pool(name="ps3p", bufs=2, space="PSUM") as ps3p,
        ):
          for layer in range(3):
            last = layer == 2

            def tsrc_view(c, _p=layer % 2):
                if c < 2:
                    return tfA[_p][c * chunk:(c + 1) * chunk, :]
                return tfB[_p][(c - 2) * chunk:(c - 1) * chunk, :]

            W_next = W2_sb if layer == 0 else W3_sb
            brow = (b1r_sb, b2r_sb, b3r_sb)[layer]
            if True:
                # gather emission leads consumption by EARLY groups; at
                # layer start the chunk-0/1 calls (fed by the already-done
                # AllGather half A) are emitted before any chunk-2/3 call so
                # the in-order GpSimd stream is not stalled by AG half B.
                EARLY = 6
                mtiles = {}
                lgroups = len(groups)

                def alloc_group(gi2):
                    for c in range(NCHUNK):
                        mt = msgp.tile([128, maxsub[c] * D], dt.float16,
                                       tag=f"msg{c}")
                        if layer == 0 and gi2 < 10:
                            # first rotation of each buffer: clear so call
                            # tails can never be NaN garbage in the matmuls
                            nc.vector.memset(mt[:], 0.0)
                        mtiles[(gi2, c)] = (mt, gdev0[(gi2, c)])

                def emit_calls(gi2, chunks):
                    nonlocal qctr
                    gcalls = {c: [cl for cl in calls
                                  if cl[0] == c and cl[1] == gi2]
                              for c in chunks}
                    mxcall = max(len(v) for v in gcalls.values())
                    for k in range(mxcall):
                        for c in chunks:
                            if k >= len(gcalls[c]):
                                continue
                            (_, _, log0, dev0, nslots) = gcalls[c][k]
                            mt, gbase = mtiles[(gi2, c)]
                            nsub = (nslots + 127) // 128
                            fo = (dev0 - gbase) // 128
                            nc.gpsimd.dma_gather(
                                mt[:, fo * D:(fo + nsub) * D]
                                .rearrange("p (s e) -> p s e", e=D),
                                tsrc_view(c),
                                idx_sb[:, log0 // 16:(log0 + nslots) // 16],
                                nslots, nslots, D, queue_num=qctr % 4)
                            qctr += 1

                for gi2 in range(min(EARLY, lgroups)):
                    alloc_group(gi2)
                for gi2 in range(min(EARLY, lgroups)):
                    emit_calls(gi2, (0, 1))
                for gi2 in range(min(EARLY, lgroups)):
                    emit_calls(gi2, (2, 3))

                for gi, (blo, bhi) in enumerate(groups):
                    if gi + EARLY < lgroups:
                        alloc_group(gi + EARLY)
                        emit_calls(gi + EARLY, (0, 1, 2, 3))

                    # --- one-hot tiles for the whole group, OHB per DVE op.
                    # Device tile order is (group, chunk, block, sub), so the
                    # group's tiles occupy contiguous dstl/disdst columns.
                    t0 = gdev0[(gi, 0)] // 128
                    t1 = (gdev0[(gi, NCHUNK - 1)] // 128
                          + gdevcols[(gi, NCHUNK - 1)])
                    ohtiles = {}
                    for tb in range(t0, t1, OHB):
                        nb = min(OHB, t1 - tb)
                        ohb = ohp.tile([128, nb, D], dt.float16, tag="ohb")
                        nc.vector.tensor_tensor(
                            ohb[:],
                            iota_sb[:].rearrange("p (s e) -> p s e", s=1)
                            .broadcast_to((128, nb, D)),
                            dstl_sb[:, tb:tb + nb]
                            .rearrange("p (s e) -> p s e", e=1)
                            .broadcast_to((128, nb, D)),
                            OP.is_equal)
                        for j in range(nb):
                            ohtiles[tb + j] = (ohb, j)

                    # --- segment-sum matmul pieces + evac per block
                    for b in range(blo, bhi):
                        st = slp.tile([128, D], dt.float16, tag="st")
                        nc.sync.dma_start(out=st[:], in_=tloc_slice(b))
                        # enumerate partition-aligned pieces over all chunks
                        pieces = []
                        for c in range(NCHUNK):
                            mt, gbase = mtiles[(gi, c)]
                            L = int(rdev[c, b]) - gbase
                            cnt = int(cap[c, b])
                            while cnt > 0:
                                p0 = L % 128
                                g = L // 128
                                # PE tile_position rule: start 0 -> up to
                                # 128 rows, start 64 -> 64, start 32/96 -> 32
                                K = min(128 if p0 == 0 else
                                        64 if p0 == 64 else 32, cnt)
                                pieces.append((mt, gbase, p0, g, K))
                                L += K
                                cnt -= K
                        ps = psp.tile([128, D], dt.float32, tag="ps")
                        for k, (mt, gbase, p0, g, K) in enumerate(pieces):
                            ohb, j = ohtiles[gbase // 128 + g]
                            oh = ohb[p0:p0 + K, j, :]
                            msl = mt[p0:p0 + K, g * D:(g + 1) * D]
                            nc.tensor.matmul(ps[:], oh, msl,
                                             start=(k == 0), stop=False)
                        nc.tensor.matmul(ps[:], ident_sb[:], st[:],
                                         start=False, stop=False)
                        # bias: ps += sqrtdeg[d] * b[f]; the dis scale at evac
                        # turns it into +b exactly (dis * sqrtdeg == 1)
                        nc.tensor.matmul(ps[:], sqd_sb[:, b * 128:(b + 1) * 128],
                                         brow[:], start=False, stop=True)
                        if last:
                            ot = evp.tile([128, D], dt.float32, tag="outsb")
                            nc.scalar.activation(ot[:], ps[:], AF.Copy,
                                                 scale=disc_sb[:, b:b + 1])
                            nc.sync.dma_start(
                                out=out_d[b * 128:(b + 1) * 128, :], in_=ot[:])
                        else:
                            h = evp.tile([128, D], dt.float16, tag="h")
                            nc.scalar.activation(h[:], ps[:], AF.Relu,
                                                 scale=disc_sb[:, b:b + 1])
                            ps3 = ps3p.tile([128, D], dt.float16, tag="ps3")
                            nc.tensor.transpose(ps3[:], h[:], ident_sb[:])
                            hT = evp.tile([128, D], dt.float16, tag="hT")
                            nc.scalar.activation(hT[:], ps3[:], AF.Copy)
                            ps2 = ps2p.tile([128, D], dt.float32, tag="ps2")
                            nc.tensor.matmul(ps2[:], hT[:], W_next[:],
                                             start=True, stop=True)
                            tn = evp.tile([128, D], dt.float16, tag="tn")
                            nc.scalar.activation(tn[:], ps2[:], AF.Copy,
                                                 scale=disc_sb[:, b:b + 1])
                            nc.sync.dma_start(out=tloc_slice(b), in_=tn[:])
                            if b == hblk - 1:
                                emit_ag(nc, mybir, (layer + 1) % 2, 0)
                if not last:
                    emit_ag(nc, mybir, (layer + 1) % 2, 1)

    nc.compile()
    return nc


# ------------------------------------------------------------------ driver


def _make_in_maps(cfg, dis, cores, inputs):
    n, nshard, shpad, nblk = (cfg[k] for k in ("n", "nshard", "shpad", "nblk"))
    x = np.asarray(inputs["x"], np.float32)
    W1 = np.asarray(inputs["W1"], f16)
    W2 = np.asarray(inputs["W2"], f16)
    W3 = np.asarray(inputs["W3"], f16)
    b1r = np.asarray(inputs["b1"], f16).reshape(1, D)
    b2r = np.asarray(inputs["b2"], f16).reshape(1, D)
    b3r = np.asarray(inputs["b3"], f16).reshape(1, D)
    iota = np.broadcast_to(np.arange(D, dtype=f16), (128, D)).copy()
    ident = np.eye(128, dtype=f16)
    ones1 = np.ones((1, D), f16)

    in_maps = []
    for ci in range(NC):
        xs = np.zeros((shpad, N_FEAT), np.float32)
        lo = min(ci * nshard, n)
        hi = min((ci + 1) * nshard, n)
        xs[:hi - lo] = x[lo:hi]
        diss = np.ones(shpad, np.float32)
        diss[:hi - lo] = dis[lo:hi]
        ca = cores[ci]
        log_total = len(ca["idx"])
        dev_total = len(ca["dstl"])
        in_maps.append({
            "xT": np.ascontiguousarray(xs.T.astype(f16)),
            "W1": W1, "W2": W2, "W3": W3,
            "b1r": b1r, "b2r": b2r, "b3r": b3r,
            "disc": np.ascontiguousarray(diss.reshape(nblk, 128).T),
            "sqd": np.ascontiguousarray((1.0 / diss).reshape(1, shpad)
                                        .astype(f16)),
            "dstl": np.ascontiguousarray(
                ca["dstl"].reshape(dev_total // 128, 128).T),
            "idx16": np.ascontiguousarray(
                np.tile(ca["idx"].reshape(log_total // 16, 16).T, (8, 1))),
            "iota": iota, "ident": ident, "ones1": ones1,
        })
    return in_maps


def run(inputs, n_nodes=N_NODES, trace=False):
    cfg = _cfg(n_nodes)
    edge_index = np.asarray(inputs["edge_index"]).astype(np.int64)
    dis, cap, rlog, rdev, calls, log_total, dev_total, cores = \
        _build_schedule(cfg, edge_index)
    nc = _build_program(cfg, cap, rlog, rdev, calls, log_total, dev_total)
    in_maps = _make_in_maps(cfg, dis, cores, inputs)

    from concourse.bass_utils import run_bass_kernel_spmd
    res = run_bass_kernel_spmd(nc, in_maps, core_ids=list(range(NC)),
                               trace=trace)
    n, nshard = cfg["n"], cfg["nshard"]
    out = np.concatenate(
        [res.results[ci]["out"][:min((ci + 1) * nshard, n) - ci * nshard]
         for ci in range(NC)], axis=0)
    return out.astype(np.float32), res


def kernel(**inputs) -> np.ndarray:
    out, _ = run(inputs)
    return out


# revision 34
# speedup vs baseline: 1.0760x; 1.0069x over previous
"""GCN encoder (3x GCNConv) Trainium2 Bass kernel, 8-core SPMD.

Strategy (dst-sharded message passing):
- Nodes dst-sharded across 8 cores (12544-row padded shards). Each core owns
  all edges (incl. self-loops) whose dst lands in its shard.
- Activations are kept as T' = dis * (H @ W) in fp16, replicated in DRAM via
  AllGather after each layer's transform.
- Propagate per core: for each 128-dst block, gather T'[src] rows via
  gpsimd.dma_gather (int16 indices => T_full split into 4 row-chunks).
  Gather calls round-robin all 4 SWDGE queues with deep msg pools so
  descriptor generation overlaps across queues (~2-4ns/desc vs 8.4 serial).
- Slot regions use RAW per-(chunk,block) capacities (max over cores, no
  128-rounding); only gather-call boundaries are padded (16 for the idx AP,
  call starts 128-aligned in the msg tile). Segment matmuls run on
  partition-offset pieces, splitting at 128-partition wraps.
- Routing tiles oh[e,d] = (dstl[e]==d) are pure 0/1, built 8 tiles per DVE
  op via broadcast APs (iota [128,1,128] is_equal dstl [128,8,1]).
- All layers accumulate psum[d,f] = oh.T @ msg (+ identity @ ownblock for
  the self-loops, + sqrtdeg x b outer product for the bias). dis[src] rides
  in T' rows; dis[dst] is the per-partition fp32 activation scale at evac:
  relu(dis*(raw + sqrtdeg*b)) == relu(dis*raw + b) since dis*sqrtdeg == 1.
- Layers 1-2 then TensorE-transpose h to hT = lhsT of the transform GEMM
  T' = dis*(h @ W); layer 3 evacuates fp32 node-major output directly.
- AllGathers are split into half-shard collectives on separate DRAM
  tensors: half A is emitted mid-propagate (after block 48), half B between
  layers, hidden behind the next layer's chunk-0/1 gathers which only need
  half A and are emitted ahead of any chunk-2/3 call.
"""

import sys
import numpy as np

for _p in ("/opt/trn_rl_repo", "/root/.axon_site/_ro/trn_rl_repo"):
    if _p not in sys.path:
        sys.path.append(_p)

N_NODES = 100000
N_FEAT = 4
D = 128
NC = 8
NCHUNK = 4
GBLK = 2  # blocks per gather group
OHB = 8  # one-hot tiles built per DVE op
MAXIDX = 1024  # max indices per dma_gather call (SWDGE desc ring capacity)

f16 = np.float16


# ---------------------------------------------------------------- host side


def _cfg(n_nodes):
    nshard = (n_nodes + NC - 1) // NC
    shpad = ((nshard + 127) // 128) * 128
    nblk = shpad // 128
    nfull = NC * shpad
    assert nfull % NCHUNK == 0
    chunk = nfull // NCHUNK
    assert chunk <= 32767 + 1  # int16 index reach (idx < chunk <= 32768)
    return dict(n=n_nodes, nshard=nshard, shpad=shpad, nblk=nblk,
                nfull=nfull, chunk=chunk)


def _groups(nblk):
    return [(g, min(g + GBLK, nblk)) for g in range(0, nblk, GBLK)]


def _build_schedule(cfg, edge_index):
    """Integer/index preprocessing.

    Slot geometry: logical slots (descriptor/idx space, call-contiguous,
    16-aligned call sizes) vs device slots (msg-tile/one-hot column space,
    128-aligned call starts; gather-call tails are never written and get
    dstl=-1 so their one-hot rows are zero).
    """
    n, nshard, shpad, nblk, chunk = (cfg[k] for k in
                                     ("n", "nshard", "shpad", "nblk", "chunk"))
    # deg/dis include the added self-loops, but the self-loop edges
    # themselves are handled on-device by a diagonal matmul against the
    # core's own T' block (affine DMA, no gather descriptors).
    deg = np.bincount(np.concatenate([edge_index[1], np.arange(n)]),
                      minlength=n).astype(np.int64)
    dis = np.where(deg > 0, 1.0 / np.sqrt(deg.astype(np.float64)), 0.0)
    src = edge_index[0].astype(np.int64)
    dst = edge_index[1].astype(np.int64)

    # row in the split T_full layout: top half-shards of all cores first
    # (tfullA = rows [0, NC*shpad/2)), then bottom half-shards (tfullB).
    half = shpad // 2
    core_of = src // nshard
    local = src % nshard
    rows = np.where(local < half,
                    core_of * half + local,
                    NC * half + core_of * half + (local - half))
    echunk = rows // chunk
    ecore = dst // nshard
    eblk = (dst % nshard) // 128
    edstl = (dst % nshard) % 128

    counts = np.zeros((NC, NCHUNK, nblk), dtype=np.int64)
    np.add.at(counts, (ecore, echunk, eblk), 1)
    # 32-aligned so every region starts on a PE quadrant boundary
    cap = np.maximum(((counts.max(axis=0) + 31) // 32) * 32, 32)

    # slot layout: group -> chunk -> packed whole regions per call.
    rlog = np.zeros((NCHUNK, nblk), dtype=np.int64)
    rdev = np.zeros((NCHUNK, nblk), dtype=np.int64)
    calls = []  # (chunk, group_index, log_off, dev_off, nslots)
    off_log = 0
    off_dev = 0

    def close_call(c, gi, log0, dev0):
        nonlocal off_log, off_dev
        pad16 = (-(off_log - log0)) % 16
        off_log += pad16
        off_dev += pad16
        nslots = off_log - log0
        if nslots:
            calls.append((c, gi, log0, dev0, nslots))
        off_dev = dev0 + ((off_dev - dev0 + 127) // 128) * 128

    for gi, (blo, bhi) in enumerate(_groups(nblk)):
        for c in range(NCHUNK):
            log0, dev0 = off_log, off_dev
            for b in range(blo, bhi):
                if off_log + int(cap[c, b]) - log0 > MAXIDX:
                    close_call(c, gi, log0, dev0)
                    log0, dev0 = off_log, off_dev
                rlog[c, b] = off_log
                rdev[c, b] = off_dev
                off_log += int(cap[c, b])
                off_dev += int(cap[c, b])
            close_call(c, gi, log0, dev0)
    log_total = off_log
    dev_total = off_dev
    assert log_total % 16 == 0 and dev_total % 128 == 0

    cores = []
    for ci in range(NC):
        m = ecore == ci
        r, ec, eb, dl, dd = rows[m], echunk[m], eblk[m], edstl[m], dst[m]
        order = np.lexsort((r, eb, ec))
        r, ec, eb, dl, dd = (a[order] for a in (r, ec, eb, dl, dd))
        key = ec * nblk + eb
        starts = np.searchsorted(key, np.arange(NCHUNK * nblk))
        ends = np.searchsorted(key, np.arange(NCHUNK * nblk), side="right")

        idx = np.zeros(log_total, np.int64)
        dstl = np.full(dev_total, -1.0, np.float64)
        disdst = np.ones(dev_total, np.float64)
        for c in range(NCHUNK):
            for b in range(nblk):
                s, e = starts[c * nblk + b], ends[c * nblk + b]
                nn = e - s
                ol, od = rlog[c, b], rdev[c, b]
                assert nn <= cap[c, b]
                idx[ol:ol + nn] = r[s:e] % chunk
                idx[ol + nn:ol + cap[c, b]] = r[e - 1] % chunk if nn else 0
                dstl[od:od + nn] = dl[s:e]
                disdst[od:od + nn] = dis[dd[s:e]]
        # call pad16 tails keep idx=0 (a valid row; their device slots have
        # dstl=-1 so the one-hot zeroes the contribution)
        cores.append(dict(idx=idx.astype(np.int16),
                          dstl=dstl.astype(f16),
                          disdst=disdst.astype(f16)))

    return dis, cap, rlog, rdev, calls, log_total, dev_total, cores


# --------------------------------------------------------------- bass build


def _build_program(cfg, cap, rlog, rdev, calls, log_total, dev_total):
    import concourse.bacc as bacc
    import concourse.tile as tile
    from concourse import mybir

    nblk, shpad, nfull, chunk = (cfg[k] for k in
                                 ("nblk", "shpad", "nfull", "chunk"))
    dt = mybir.dt
    AF = mybir.ActivationFunctionType
    OP = mybir.AluOpType
    S_dev = dev_total // 128
    idxcols = log_total // 16
    groups = _groups(nblk)

    nc = bacc.Bacc("TRN2", target_bir_lowering=False, debug=False,
                   num_devices=NC, num_swdge_queues=4)

    # --- I/O
    xT_d = nc.dram_tensor("xT", [N_FEAT, shpad], dt.float16, kind="ExternalInput")
    W1_d = nc.dram_tensor("W1", [N_FEAT, D], dt.float16, kind="ExternalInput")
    W2_d = nc.dram_tensor("W2", [D, D], dt.float16, kind="ExternalInput")
    W3_d = nc.dram_tensor("W3", [D, D], dt.float16, kind="ExternalInput")
    b1r_d = nc.dram_tensor("b1r", [1, D], dt.float16, kind="ExternalInput")
    b2r_d = nc.dram_tensor("b2r", [1, D], dt.float16, kind="ExternalInput")
    b3r_d = nc.dram_tensor("b3r", [1, D], dt.float16, kind="ExternalInput")
    disc_d = nc.dram_tensor("disc", [128, nblk], dt.float32, kind="ExternalInput")
    sqd_d = nc.dram_tensor("sqd", [1, shpad], dt.float16, kind="ExternalInput")
    dstl_d = nc.dram_tensor("dstl", [128, S_dev], dt.float16, kind="ExternalInput")
    idx_d = nc.dram_tensor("idx16", [128, idxcols], dt.int16, kind="ExternalInput")
    iota_d = nc.dram_tensor("iota", [128, D], dt.float16, kind="ExternalInput")
    ident_d = nc.dram_tensor("ident", [128, D], dt.float16, kind="ExternalInput")
    ones_d = nc.dram_tensor("ones1", [1, D], dt.float16, kind="ExternalInput")
    out_d = nc.dram_tensor("out", [shpad, D], dt.float32, kind="ExternalOutput")

    # internal DRAM: allgather bounce + double-buffered replicated T', split
    # into top/bottom half-shard tensors so each half's AllGather can be
    # emitted as soon as its source blocks are evacuated and the next
    # layer's gathers can start on the half that's ready.
    half = shpad // 2
    hblk = half // 128
    tlocA = nc.dram_tensor("t_locA", [half, D], dt.float16)
    tlocB = nc.dram_tensor("t_locB", [half, D], dt.float16)
    tfA = [nc.dram_tensor(f"t_fullA{i}", [NC * half, D], dt.float16)
           for i in range(2)]
    tfB = [nc.dram_tensor(f"t_fullB{i}", [NC * half, D], dt.float16)
           for i in range(2)]

    def tloc_slice(b):
        if b < hblk:
            return tlocA[b * 128:(b + 1) * 128, :]
        return tlocB[(b - hblk) * 128:(b - hblk + 1) * 128, :]

    def emit_ag(nc_, mybir_, parity, which):
        src = tlocA if which == 0 else tlocB
        dst = (tfA if which == 0 else tfB)[parity]
        nc_.gpsimd.collective_compute(
            "AllGather", mybir_.AluOpType.bypass,
            replica_groups=[list(range(NC))],
            ins=[src[:, :].opt()], outs=[dst[:, :].opt()])

    # per-(group, chunk) device-column extents for msg tiles
    gdev0 = {}
    gdevcols = {}
    for (c, gi, log0, dev0, nslots) in calls:
        k = (gi, c)
        if k not in gdev0:
            gdev0[k] = dev0
        gdevcols[k] = (dev0 - gdev0[k]) // 128 + (nslots + 127) // 128
    maxsub = {c: max(v for (gi, cc), v in gdevcols.items() if cc == c)
              for c in range(NCHUNK)}

    from contextlib import ExitStack
    with tile.TileContext(nc) as tc, ExitStack() as stack:
        # ---- resident tiles (pool stays open for the whole program)
        res = stack.enter_context(tc.tile_pool(name="res", bufs=1))
        idx_sb = res.tile([128, idxcols], dt.int16, tag="idx")
        dstl_sb = res.tile([128, S_dev], dt.float16, tag="dstl")
        sqd_sb = res.tile([1, shpad], dt.float16, tag="sqd")
        disc_sb = res.tile([128, nblk], dt.float32, tag="disc")
        iota_sb = res.tile([128, D], dt.float16, tag="iota")
        ident_sb = res.tile([128, D], dt.float16, tag="ident")
        ones_sb = res.tile([1, D], dt.float16, tag="ones")
        xT_sb = res.tile([N_FEAT, shpad], dt.float16, tag="xT")
        W1_sb = res.tile([N_FEAT, D], dt.float16, tag="W1")
        W2_sb = res.tile([D, D], dt.float16, tag="W2")
        W3_sb = res.tile([D, D], dt.float16, tag="W3")
        b1r_sb = res.tile([1, D], dt.float16, tag="b1r")
        b2r_sb = res.tile([1, D], dt.float16, tag="b2r")
        b3r_sb = res.tile([1, D], dt.float16, tag="b3r")

        for sb, d in ((idx_sb, idx_d), (dstl_sb, dstl_d), (sqd_sb, sqd_d),
                      (disc_sb, disc_d), (iota_sb, iota_d), (ident_sb, ident_d),
                      (ones_sb, ones_d), (xT_sb, xT_d), (W1_sb, W1_d),
                      (W2_sb, W2_d), (W3_sb, W3_d), (b1r_sb, b1r_d),
                      (b2r_sb, b2r_d), (b3r_sb, b3r_d)):
            nc.sync.dma_start(out=sb[:], in_=d[:, :])

        # ---- layer 1 transform: T1' = dis * (x @ W1) -> tloc, allgather
        with (
            tc.tile_pool(name="p1ps", bufs=4, space="PSUM") as p1ps,
            tc.tile_pool(name="p1sb", bufs=4) as p1sb,
        ):
            for b in range(nblk):
                ps = p1ps.tile([128, D], dt.float32, tag="t1ps")
                nc.tensor.matmul(ps[:], xT_sb[:, b * 128:(b + 1) * 128],
                                 W1_sb[:], start=True, stop=True)
                t1 = p1sb.tile([128, D], dt.float16, tag="t1sb")
                nc.scalar.activation(t1[:], ps[:], AF.Copy,
                                     scale=disc_sb[:, b:b + 1])
                nc.sync.dma_start(out=tloc_slice(b), in_=t1[:])
                if b == hblk - 1:
                    emit_ag(nc, mybir, 0, 0)
        emit_ag(nc, mybir, 0, 1)

        # ---- layers
        qctr = 0
        EARLY = 6
        mtiles = {}
        allocated = set()
        emitted01 = set()
        lgroups = len(groups)
        with (
            tc.tile_pool(name="msgp", bufs=10) as msgp,
            tc.tile_pool(name="ohp", bufs=6) as ohp,

            tc.tile_pool(name="evp", bufs=4) as evp,
            tc.tile_pool(name="slp", bufs=4) as slp,
            tc.tile_pool(name="psp", bufs=4, space="PSUM") as psp,
            tc.tile_pool(name="ps2p", bufs=2, space="PSUM") as ps2p,
            tc.tile_pool(name="ps3p", bufs=2, space="PSUM") as ps3p,
        ):
          def tsrc_view(c, p):
              if c < 2:
                  return tfA[p][c * chunk:(c + 1) * chunk, :]
              return tfB[p][(c - 2) * chunk:(c - 1) * chunk, :]

          def alloc_group(tl, gi2):
              for c in range(NCHUNK):
                  mt = msgp.tile([128, maxsub[c] * D], dt.float16,
                                 tag=f"msg{c}")
                  if tl == 0 and gi2 < 10:
                      # first rotation of each buffer: clear so call tails
                      # can never be NaN garbage in the matmuls
                      nc.vector.memset(mt[:], 0.0)
                  mtiles[(tl, gi2, c)] = (mt, gdev0[(gi2, c)])
              allocated.add((tl, gi2))

          def emit_calls(tl, gi2, chunks):
              nonlocal qctr
              gcalls = {c: [cl for cl in calls
                            if cl[0] == c and cl[1] == gi2]
                        for c in chunks}
              mxcall = max(len(v) for v in gcalls.values())
              for k in range(mxcall):
                  for c in chunks:
                      if k >= len(gcalls[c]):
                          continue
                      (_, _, log0, dev0, nslots) = gcalls[c][k]
                      mt, gbase = mtiles[(tl, gi2, c)]
                      nsub = (nslots + 127) // 128
                      fo = (dev0 - gbase) // 128
                      nc.gpsimd.dma_gather(
                          mt[:, fo * D:(fo + nsub) * D]
                          .rearrange("p (s e) -> p s e", e=D),
                          tsrc_view(c, tl % 2),
                          idx_sb[:, log0 // 16:(log0 + nslots) // 16],
                          nslots, nslots, D, queue_num=qctr % 4)
                      qctr += 1
              if tuple(chunks) == (0, 1):
                  emitted01.add((tl, gi2))

          for layer in range(3):
            last = layer == 2

            W_next = W2_sb if layer == 0 else W3_sb
            brow = (b1r_sb, b2r_sb, b3r_sb)[layer]
            if True:
                # gather emission leads consumption by EARLY groups; the
                # chunk-0/1 calls of the NEXT layer's first groups are
                # pre-emitted at the end of this layer (they only need
                # AllGather half A, emitted mid-propagate), so the in-order
                # GpSimd stream never idles across layer boundaries.
                for gi2 in range(min(EARLY, lgroups)):
                    if (layer, gi2) not in allocated:
                        alloc_group(layer, gi2)
                for gi2 in range(min(EARLY, lgroups)):
                    if (layer, gi2) not in emitted01:
                        emit_calls(layer, gi2, (0, 1))
                for gi2 in range(min(EARLY, lgroups)):
                    emit_calls(layer, gi2, (2, 3))

                for gi, (blo, bhi) in enumerate(groups):
                    if gi + EARLY < lgroups:
                        alloc_group(layer, gi + EARLY)
                        emit_calls(layer, gi + EARLY, (0, 1, 2, 3))

                    # --- one-hot tiles for the whole group, OHB per DVE op.
                    # Device tile order is (group, chunk, block, sub), so the
                    # group's tiles occupy contiguous dstl/disdst columns.
                    t0 = gdev0[(gi, 0)] // 128
                    t1 = (gdev0[(gi, NCHUNK - 1)] // 128
                          + gdevcols[(gi, NCHUNK - 1)])
                    ohtiles = {}
                    for tb in range(t0, t1, OHB):
                        nb = min(OHB, t1 - tb)
                        ohb = ohp.tile([128, nb, D], dt.float16, tag="ohb")
                        nc.vector.tensor_tensor(
                            ohb[:],
                            iota_sb[:].rearrange("p (s e) -> p s e", s=1)
                            .broadcast_to((128, nb, D)),
                            dstl_sb[:, tb:tb + nb]
                            .rearrange("p (s e) -> p s e", e=1)
                            .broadcast_to((128, nb, D)),
                            OP.is_equal)
                        for j in range(nb):
                            ohtiles[tb + j] = (ohb, j)

                    # --- segment-sum matmul pieces + evac per block
                    for b in range(blo, bhi):
                        st = slp.tile([128, D], dt.float16, tag="st")
                        nc.sync.dma_start(out=st[:], in_=tloc_slice(b))
                        # enumerate partition-aligned pieces over all chunks
                        pieces = []
                        for c in range(NCHUNK):
                            mt, gbase = mtiles[(layer, gi, c)]
                            L = int(rdev[c, b]) - gbase
                            cnt = int(cap[c, b])
                            while cnt > 0:
                                p0 = L % 128
                                g = L // 128
                                # PE tile_position rule: start 0 -> up to
                                # 128 rows, start 64 -> 64, start 32/96 -> 32
                                K = min(128 if p0 == 0 else
                                        64 if p0 == 64 else 32, cnt)
                                pieces.append((mt, gbase, p0, g, K))
                                L += K
                                cnt -= K
                        ps = psp.tile([128, D], dt.float32, tag="ps")
                        for k, (mt, gbase, p0, g, K) in enumerate(pieces):
                            ohb, j = ohtiles[gbase // 128 + g]
                            oh = ohb[p0:p0 + K, j, :]
                            msl = mt[p0:p0 + K, g * D:(g + 1) * D]
                            nc.tensor.matmul(ps[:], oh, msl,
                                             start=(k == 0), stop=False)
                        nc.tensor.matmul(ps[:], ident_sb[:], st[:],
                                         start=False, stop=False)
                        # bias: ps += sqrtdeg[d] * b[f]; the dis scale at evac
                        # turns it into +b exactly (dis * sqrtdeg == 1)
                        nc.tensor.matmul(ps[:], sqd_sb[:, b * 128:(b + 1) * 128],
                                         brow[:], start=False, stop=True)
                        if last:
                            ot = evp.tile([128, D], dt.float32, tag="outsb")
                            nc.scalar.activation(ot[:], ps[:], AF.Copy,
                                                 scale=disc_sb[:, b:b + 1])
                            nc.sync.dma_start(
                                out=out_d[b * 128:(b + 1) * 128, :], in_=ot[:])
                        else:
                            h = evp.tile([128, D], dt.float16, tag="h")
                            nc.scalar.activation(h[:], ps[:], AF.Relu,
                                                 scale=disc_sb[:, b:b + 1])
                            ps3 = ps3p.tile([128, D], dt.float16, tag="ps3")
                            nc.tensor.transpose(ps3[:], h[:], ident_sb[:])
                            hT = evp.tile([128, D], dt.float16, tag="hT")
                            nc.scalar.activation(hT[:], ps3[:], AF.Copy)
                            ps2 = ps2p.tile([128, D], dt.float32, tag="ps2")
                            nc.tensor.matmul(ps2[:], hT[:], W_next[:],
                                             start=True, stop=True)
                            tn = evp.tile([128, D], dt.float16, tag="tn")
                            nc.scalar.activation(tn[:], ps2[:], AF.Copy,
                                                 scale=disc_sb[:, b:b + 1])
                            nc.sync.dma_start(out=tloc_slice(b), in_=tn[:])
                            if b == hblk - 1:
                                emit_ag(nc, mybir, (layer + 1) % 2, 0)
                if not last:
                    for gi2 in range(min(EARLY, lgroups)):
                        alloc_group(layer + 1, gi2)
                        emit_calls(layer + 1, gi2, (0, 1))
                    emit_ag(nc, mybir, (layer + 1) % 2, 1)

    nc.compile()
    return nc


# ------------------------------------------------------------------ driver


def _make_in_maps(cfg, dis, cores, inputs):
    n, nshard, shpad, nblk = (cfg[k] for k in ("n", "nshard", "shpad", "nblk"))
    x = np.asarray(inputs["x"], np.float32)
    W1 = np.asarray(inputs["W1"], f16)
    W2 = np.asarray(inputs["W2"], f16)
    W3 = np.asarray(inputs["W3"], f16)
    b1r = np.asarray(inputs["b1"], f16).reshape(1, D)
    b2r = np.asarray(inputs["b2"], f16).reshape(1, D)
    b3r = np.asarray(inputs["b3"], f16).reshape(1, D)
    iota = np.broadcast_to(np.arange(D, dtype=f16), (128, D)).copy()
    ident = np.eye(128, dtype=f16)
    ones1 = np.ones((1, D), f16)

    in_maps = []
    for ci in range(NC):
        xs = np.zeros((shpad, N_FEAT), np.float32)
        lo = min(ci * nshard, n)
        hi = min((ci + 1) * nshard, n)
        xs[:hi - lo] = x[lo:hi]
        diss = np.ones(shpad, np.float32)
        diss[:hi - lo] = dis[lo:hi]
        ca = cores[ci]
        log_total = len(ca["idx"])
        dev_total = len(ca["dstl"])
        in_maps.append({
            "xT": np.ascontiguousarray(xs.T.astype(f16)),
            "W1": W1, "W2": W2, "W3": W3,
            "b1r": b1r, "b2r": b2r, "b3r": b3r,
            "disc": np.ascontiguousarray(diss.reshape(nblk, 128).T),
            "sqd": np.ascontiguousarray((1.0 / diss).reshape(1, shpad)
                                        .astype(f16)),
            "dstl": np.ascontiguousarray(
                ca["dstl"].reshape(dev_total // 128, 128).T),
            "idx16": np.ascontiguousarray(
                np.tile(ca["idx"].reshape(log_total // 16, 16).T, (8, 1))),
            "iota": iota, "ident": ident, "ones1": ones1,
        })
    return in_maps


def run(inputs, n_nodes=N_NODES, trace=False):
    cfg = _cfg(n_nodes)
    edge_index = np.asarray(inputs["edge_index"]).astype(np.int64)
    dis, cap, rlog, rdev, calls, log_total, dev_total, cores = \
        _build_schedule(cfg, edge_index)
    nc = _build_program(cfg, cap, rlog, rdev, calls, log_total, dev_total)
    in_maps = _make_in_maps(cfg, dis, cores, inputs)

    from concourse.bass_utils import run_bass_kernel_spmd
    res = run_bass_kernel_spmd(nc, in_maps, core_ids=list(range(NC)),
                               trace=trace)
    n, nshard = cfg["n"], cfg["nshard"]
    out = np.concatenate(
        [res.results[ci]["out"][:min((ci + 1) * nshard, n) - ci * nshard]
         for ci in range(NC)], axis=0)
    return out.astype(np.float32), res


def kernel(**inputs) -> np.ndarray:
    out, _ = run(inputs)
    return out
